# revision 9
# baseline (speedup 1.0000x reference)
"""AlphaFold-style OuterProductMean pair feature on 8 trn2 NeuronCores.

Computation (full shapes):
    x_left, x_right: (1, N=128, R=256, E=32) fp32
    outer[b,i,j,l,r] = sum_n x_left[b,n,i,l] * x_right[b,n,j,r]
    out = outer.reshape(1, R, R, E*E) @ W + b          # W: (1024, 128)

Sharding: row-shard the pair grid — core k owns i in [32k, 32k+32).
Each core receives its x_left row block, the full x_right, W, b
(all staged host-side; no collectives), and writes its (32, 256, 128)
output row block. Host concatenates.

Per-core kernel:
  stage 1 (bf16 matmuls, 1 cycle/row): for each i, r:
           outer_i[l, j] = xl[:, i, :].T @ xr[:, :, r]   (K=n=128)
           Four r's run concurrently via 4x column tiling (M=32 each)
           producing PSUM chunks (128 part = (r%4, l), 256 j) whose
           partition index matches rows r*32+l of the host-permuted W.
  stage 2: pair[d, (u j)] = sum_c Wp_chunk[c].T @ outer_chunk[c]
           (8 accumulating matmuls, K=128, N=512 = two i's of 256 j).
Output per core is (i, d, j); host transposes to (i, j, d).
"""

import sys

if "/opt/trn_rl_repo" not in sys.path:
    sys.path.insert(0, "/opt/trn_rl_repo")

from contextlib import ExitStack

import numpy as np

import concourse.bass as bass
import concourse.tile as tile
from concourse import bacc, mybir
from concourse.bass_utils import run_bass_kernel_spmd

N_CORES = 8
N = 128  # MSA depth (contraction dim)
R = 256  # residues
E = 32   # 1D embedding
D = 128  # 2D embedding
IB = R // N_CORES  # 32 rows of i per core
BENCH_REPS = 5
STAGE2_BF16 = True  # False -> float32r stage 2 (more precise, maybe slower)

_cached = None
last_results = None  # BassKernelResults of the most recent run (for test harness)


def _build(reps=1, stage2_bf16=STAGE2_BF16):
    f32 = mybir.dt.float32
    f32r = mybir.dt.float32r
    bf16 = mybir.dt.bfloat16
    s2dt = bf16 if stage2_bf16 else f32r

    nc = bacc.Bacc(None, target_bir_lowering=False, debug=False)

    xl_d = nc.dram_tensor("xl", [N, IB * E], bf16, kind="ExternalInput")    # [n, i*32+l]
    xr_d = nc.dram_tensor("xr", [N, E * R], bf16, kind="ExternalInput")     # [n, r*256+j]
    wp_d = nc.dram_tensor("wp", [D, 8 * D], s2dt, kind="ExternalInput")     # [p, c*128+d]
    b_d = nc.dram_tensor("bias", [D, 1], f32, kind="ExternalInput")
    out_d = nc.dram_tensor("out", [IB, D, R], f32, kind="ExternalOutput")   # [i, d, j]

    with tile.TileContext(nc) as tc, ExitStack() as ctx:
        const = ctx.enter_context(tc.tile_pool(name="const", bufs=1))
        xl_sb = const.tile([N, IB * E], bf16)
        xr_sb = const.tile([N, E * R], bf16)
        wp_sb = const.tile([D, 8 * D], s2dt)
        b_sb = const.tile([D, 1], f32)

        nc.sync.dma_start(xl_sb[:], xl_d[:])
        for q in range(8):
            s = q * (E * R // 8)
            w = E * R // 8
            nc.sync.dma_start(xr_sb[:, s:s + w], xr_d[:, s:s + w])
        nc.sync.dma_start(wp_sb[:], wp_d[:])
        nc.sync.dma_start(b_sb[:], b_d[:])

        outer_pool = ctx.enter_context(tc.tile_pool(name="outer", bufs=2))
        ps1 = ctx.enter_context(tc.tile_pool(name="ps1", bufs=4, space="PSUM"))
        ps2 = ctx.enter_context(tc.tile_pool(name="ps2", bufs=2, space="PSUM"))
        osb_pool = ctx.enter_context(tc.tile_pool(name="osb", bufs=3))

        evac_idx = 0
        for _rep in range(reps):
            for ip in range(IB // 2):  # pairs of i rows
                outer = outer_pool.tile([D, 8, 2, R], s2dt)  # (p, c, u, j)
                for u in range(2):
                    i = 2 * ip + u
                    for cp in range(4):  # chunk pairs share one PSUM bank
                        p1 = ps1.tile([D, 2, R], f32)
                        for q in range(2):
                            c = 2 * cp + q
                            for g in range(4):
                                r0 = 4 * c + g
                                nc.tensor.matmul(
                                    p1[32 * g:32 * g + 32, q, :],
                                    xl_sb[:, E * i:E * i + E],
                                    xr_sb[:, R * r0:R * r0 + R],
                                    start=True,
                                    stop=True,
                                    tile_position=(0, 32 * g),
                                )
                        # split PSUM evacuation across DVE and ACT (5:3 —
                        # DVE ~533ns/tile also carries the bias adds, ACT
                        # ~2x slower per copy; nc.any routes all to ACT)
                        dst = outer[:, 2 * cp:2 * cp + 2, u, :]
                        if evac_idx % 8 < 5:
                            nc.vector.tensor_copy(dst, p1[:])
                        else:
                            nc.scalar.copy(dst, p1[:])
                        evac_idx += 1

                p2 = ps2.tile([D, 2, R], f32)
                for c in range(8):
                    nc.tensor.matmul(
                        p2[:],
                        wp_sb[:, D * c:D * c + D],
                        outer[:, c],
                        start=(c == 0),
                        stop=(c == 7),
                    )
                osb = osb_pool.tile([D, 2, R], f32)
                nc.vector.tensor_scalar_add(osb[:], p2[:], b_sb[:])
                nc.sync.dma_start(out_d[2 * ip], osb[:, 0, :])
                nc.sync.dma_start(out_d[2 * ip + 1], osb[:, 1, :])

    nc.compile()
    return nc


def make_in_maps(x_left, x_right, W, b, stage2_bf16=STAGE2_BF16):
    import ml_dtypes

    xl = np.asarray(x_left, dtype=np.float32)[0]   # (n, i, l)
    xr = np.asarray(x_right, dtype=np.float32)[0]  # (n, j, r)
    W = np.asarray(W, dtype=np.float32)
    b = np.asarray(b, dtype=np.float32)

    xl = np.ascontiguousarray(xl).astype(ml_dtypes.bfloat16)
    xr_flat = np.ascontiguousarray(
        xr.transpose(0, 2, 1).astype(ml_dtypes.bfloat16)
    ).reshape(N, E * R)  # [n, r*256+j]
    # W[(l*32+r), d] -> W_perm[(r*32+l), d] -> chunk-major sbuf layout [p, c*128+d]
    wp = (
        W.reshape(E, E, D).transpose(1, 0, 2).reshape(8, D, D)
        .transpose(1, 0, 2).reshape(D, 8 * D)
    )
    wp = np.ascontiguousarray(wp)
    if stage2_bf16:
        wp = wp.astype(ml_dtypes.bfloat16)
    bias = np.ascontiguousarray(b.reshape(D, 1))

    in_maps = []
    for k in range(N_CORES):
        xlk = np.ascontiguousarray(xl[:, IB * k:IB * (k + 1), :]).reshape(N, IB * E)
        in_maps.append({"xl": xlk, "xr": xr_flat, "wp": wp, "bias": bias})
    return in_maps


def kernel(x_left, x_right, W, b):
    global _cached, last_results
    if _cached is None:
        _cached = _build()
    nc = _cached

    in_maps = make_in_maps(x_left, x_right, W, b)
    res = run_bass_kernel_spmd(nc, in_maps, list(range(N_CORES)))
    last_results = res

    blocks = [res.results[k]["out"].transpose(0, 2, 1) for k in range(N_CORES)]
    return np.concatenate(blocks, axis=0)[None]  # (1, 256, 256, 128)
